# revision 3
# baseline (speedup 1.0000x reference)
"""7x7 'same' 2D convolution over [128, 512, 512] f32, data-parallel on 8 NeuronCores.

Formulation: for each output-row block of M=122 rows, the row-direction
(u-tap) contraction is a banded Toeplitz matmul on the TensorEngine:
    out[i0+m, j] = sum_v sum_r T_v[r, m] * xpad[i0+r, j+v]
with T_v[r, m] = w[r-m, v] (band 0 <= r-m < 7). The 7 column taps (v)
are 7 matmuls accumulating into the same PSUM bank, each reading the
same SBUF x-tile at a shifted column offset. Inputs are cast to fp16
host-side (full-rate on the PE, ~1e-3 rel err); accumulation is fp32.

DMA strategy: one staging tile per image holds the 5 overlapping
128-row chunks (row stride 122 between chunks) so each image loads with
2 big DMAs on the sync HWDGE ring; outputs collect into one [128,5,512]
tile and store with 2 big DMAs per image on the scalar HWDGE ring.
"""

import numpy as np

B, H, W = 128, 512, 512
KS = 7
PAD = (KS - 1) // 2          # 3
HP = H + 2 * PAD             # 518
N_CORES = 8
PER_CORE = B // N_CORES      # 16
MBLK = 128 - (KS - 1)        # 122 output rows per full block
NBLK = 5                     # ceil(512 / 122); last block has 24 rows


def _build_program():
    import concourse.bass as bass
    import concourse.tile as tile
    from concourse import bacc, mybir

    f16 = mybir.dt.float16
    f32 = mybir.dt.float32

    nc = bacc.Bacc("TRN2", target_bir_lowering=False, debug=False,
                   num_devices=N_CORES)
    x_ext = nc.declare_dram_parameter("x", [PER_CORE, HP, HP], f16,
                                      isOutput=False)
    t_ext = nc.declare_dram_parameter("toep", [128, KS * MBLK], f16,
                                      isOutput=False)
    out_ext = nc.declare_dram_parameter("out", [PER_CORE, H, W], f32,
                                        isOutput=True)

    with tile.TileContext(nc) as tc:
        with (
            tc.tile_pool(name="toep", bufs=1) as toep_pool,
            tc.tile_pool(name="xin", bufs=3) as x_pool,
            tc.tile_pool(name="psum", bufs=6, space="PSUM") as psum_pool,
            tc.tile_pool(name="outs", bufs=2) as out_pool,
        ):
            toep_sb = toep_pool.tile([128, KS * MBLK], f16)
            nc.sync.dma_start(out=toep_sb[:], in_=t_ext[:])

            for img in range(PER_CORE):
                # Stage the image as 5 overlapping 128-row chunks:
                # chunk c holds padded rows [122c, 122c+128).
                stage = x_pool.tile([128, NBLK, HP], f16)
                src = bass.AP(
                    x_ext,
                    img * HP * HP,
                    [(HP, 128), (MBLK * HP, 4), (1, HP)],
                )
                nc.sync.dma_start(out=stage[:, 0:4, :], in_=src)
                # chunk 4: padded rows [488, 518) (30 rows; rest unused)
                nc.sync.dma_start(out=stage[:30, 4, :],
                                  in_=x_ext[img, 4 * MBLK:HP, :])

                o_sb = out_pool.tile([128, NBLK, W], f32)
                for b in range(NBLK):
                    m = MBLK if b < 4 else H - 4 * MBLK      # 122 / 24
                    kin = 128 if b < 4 else HP - 4 * MBLK    # 128 / 30
                    psum = psum_pool.tile([128, W], f32)
                    for v in range(KS):
                        nc.tensor.matmul(
                            psum[:m, :],
                            toep_sb[:kin, v * MBLK:v * MBLK + m],
                            stage[:kin, b, v:v + W],
                            start=(v == 0),
                            stop=(v == KS - 1),
                        )
                    nc.vector.tensor_copy(o_sb[:m, b, :], psum[:m, :])

                dst = out_ext[img, 0:4 * MBLK, :].rearrange(
                    "(c p) w -> p c w", p=MBLK)
                nc.scalar.dma_start(out=dst, in_=o_sb[:MBLK, 0:4, :])
                nc.scalar.dma_start(out=out_ext[img, 4 * MBLK:H, :],
                                    in_=o_sb[:H - 4 * MBLK, 4, :])
    nc.finalize()
    return nc


def _host_prep(x, w):
    x = np.asarray(x, dtype=np.float32)
    w = np.asarray(w, dtype=np.float32)
    xpad = np.zeros((B, HP, HP), dtype=np.float16)
    xpad[:, PAD:PAD + H, PAD:PAD + W] = x
    toep = np.zeros((128, KS * MBLK), dtype=np.float16)
    w16 = w.astype(np.float16)
    idx = np.arange(MBLK)
    for v in range(KS):
        for d in range(KS):
            toep[idx + d, v * MBLK + idx] = w16[d, v]
    return xpad, toep


def kernel(x, w):
    from concourse.bass_utils import run_bass_kernel_spmd

    xpad, toep = _host_prep(x, w)
    nc = _build_program()
    in_maps = [
        {"x": xpad[c * PER_CORE:(c + 1) * PER_CORE], "toep": toep}
        for c in range(N_CORES)
    ]
    res = run_bass_kernel_spmd(nc, in_maps, core_ids=list(range(N_CORES)))
    return np.concatenate(
        [res.results[c]["out"] for c in range(N_CORES)], axis=0
    )


# revision 6
# speedup vs baseline: 1.7860x; 1.7860x over previous
"""7x7 'same' 2D convolution over [128, 512, 512] f32, data-parallel on 8 NeuronCores.

Formulation: for each output-row block of M=122 rows, the row-direction
(u-tap) contraction is a banded Toeplitz matmul on the TensorEngine:
    out[i0+m, j] = sum_v sum_r T_v[r, m] * xpad[i0+r, j+v]
with T_v[r, m] = w[r-m, v] (band 0 <= r-m < 7). The 7 column taps (v)
are 7 matmuls accumulating into the same PSUM bank, each reading the
same SBUF x-tile at a shifted column offset. Inputs are cast to fp16
host-side (full-rate on the PE, ~1e-3 rel err); accumulation is fp32;
outputs are stored as bf16 and upcast on the host.

DMA strategy: one staging tile per image holds the 5 overlapping
128-row chunks (row stride 122 between chunks) so each image loads with
2 big DMAs; outputs collect into one [128,5,512] tile and store with 2
DMAs per image into a (p, chunk)-major DRAM layout so the writes walk
DRAM sequentially (strided interleaved writes measured ~25x slower).
The host un-permutes the chunked output. Loads and stores alternate
between the two HWDGE rings (sync / scalar) per image.
"""

import numpy as np

B, H, W = 128, 512, 512
KS = 7
PAD = (KS - 1) // 2          # 3
HP = H + 2 * PAD             # 518
N_CORES = 8
PER_CORE = B // N_CORES      # 16
MBLK = 128 - (KS - 1)        # 122 output rows per full block
NBLK = 5                     # ceil(512 / 122); last block has 24 rows
MRUNT = H - 4 * MBLK         # 24
KRUNT = HP - 4 * MBLK        # 30


def _build_program():
    import concourse.bass as bass
    import concourse.tile as tile
    from concourse import bacc, mybir

    f16 = mybir.dt.float16
    bf16 = mybir.dt.bfloat16
    f32 = mybir.dt.float32

    nc = bacc.Bacc("TRN2", target_bir_lowering=False, debug=False,
                   num_devices=N_CORES)
    x_ext = nc.declare_dram_parameter("x", [PER_CORE, HP, HP], f16,
                                      isOutput=False)
    t_ext = nc.declare_dram_parameter("toep", [128, KS * MBLK], f16,
                                      isOutput=False)
    # p-major chunked output: opc[img, p, c, :] = out row 122c + p
    opc_ext = nc.declare_dram_parameter("opc", [PER_CORE, MBLK, 4, W],
                                        bf16, isOutput=True)
    ort_ext = nc.declare_dram_parameter("ort", [PER_CORE, MRUNT, W],
                                        bf16, isOutput=True)

    with tile.TileContext(nc) as tc:
        with (
            tc.tile_pool(name="toep", bufs=1) as toep_pool,
            tc.tile_pool(name="xin", bufs=3) as x_pool,
            tc.tile_pool(name="psum", bufs=6, space="PSUM") as psum_pool,
            tc.tile_pool(name="outs", bufs=3) as out_pool,
        ):
            toep_sb = toep_pool.tile([128, KS * MBLK], f16)
            nc.sync.dma_start(out=toep_sb[:], in_=t_ext[:])

            for img in range(PER_CORE):
                ld = nc.sync if img % 2 == 0 else nc.scalar
                st = nc.scalar if img % 2 == 0 else nc.sync
                # Stage the image as 5 overlapping 128-row chunks:
                # chunk c holds padded rows [122c, 122c+128).
                stage = x_pool.tile([128, NBLK, HP], f16)
                src = bass.AP(
                    x_ext,
                    img * HP * HP,
                    [(HP, 128), (MBLK * HP, 4), (1, HP)],
                )
                ld.dma_start(out=stage[:, 0:4, :], in_=src)
                # chunk 4: padded rows [488, 518) (30 rows; rest unused)
                ld.dma_start(out=stage[:KRUNT, 4, :],
                             in_=x_ext[img, 4 * MBLK:HP, :])

                o_sb = out_pool.tile([128, NBLK, W], bf16)
                for b in range(NBLK):
                    m = MBLK if b < 4 else MRUNT
                    kin = 128 if b < 4 else KRUNT
                    psum = psum_pool.tile([128, W], f32)
                    for v in range(KS):
                        nc.tensor.matmul(
                            psum[:m, :],
                            toep_sb[:kin, v * MBLK:v * MBLK + m],
                            stage[:kin, b, v:v + W],
                            start=(v == 0),
                            stop=(v == KS - 1),
                        )
                    nc.vector.tensor_copy(o_sb[:m, b, :], psum[:m, :])

                st.dma_start(out=opc_ext[img], in_=o_sb[:MBLK, 0:4, :])
                st.dma_start(out=ort_ext[img], in_=o_sb[:MRUNT, 4, :])
    nc.finalize()
    return nc


def _host_prep(x, w):
    x = np.asarray(x, dtype=np.float32)
    w = np.asarray(w, dtype=np.float32)
    xpad = np.zeros((B, HP, HP), dtype=np.float16)
    xpad[:, PAD:PAD + H, PAD:PAD + W] = x
    toep = np.zeros((128, KS * MBLK), dtype=np.float16)
    w16 = w.astype(np.float16)
    idx = np.arange(MBLK)
    for v in range(KS):
        for d in range(KS):
            toep[idx + d, v * MBLK + idx] = w16[d, v]
    return xpad, toep


def _execute(x, w, **run_kwargs):
    from concourse.bass_utils import run_bass_kernel_spmd

    xpad, toep = _host_prep(x, w)
    nc = _build_program()
    in_maps = [
        {"x": xpad[c * PER_CORE:(c + 1) * PER_CORE], "toep": toep}
        for c in range(N_CORES)
    ]
    res = run_bass_kernel_spmd(nc, in_maps, core_ids=list(range(N_CORES)),
                               **run_kwargs)
    out = np.empty((B, H, W), dtype=np.float32)
    for c in range(N_CORES):
        sl = slice(c * PER_CORE, (c + 1) * PER_CORE)
        opc = np.asarray(res.results[c]["opc"], dtype=np.float32)
        ort = np.asarray(res.results[c]["ort"], dtype=np.float32)
        # opc[img, p, c, :] -> rows (c*MBLK + p); c-major then p
        out[sl, :4 * MBLK, :] = opc.transpose(0, 2, 1, 3).reshape(
            PER_CORE, 4 * MBLK, W)
        out[sl, 4 * MBLK:, :] = ort
    return out, res


def kernel(x, w):
    out, _ = _execute(x, w)
    return out


# revision 10
# speedup vs baseline: 1.7875x; 1.0008x over previous
"""7x7 'same' 2D convolution over [128, 512, 512] f32, data-parallel on 8 NeuronCores.

Formulation: for each output-row block of M=122 rows, the row-direction
(u-tap) contraction is a banded Toeplitz matmul on the TensorEngine:
    out[i0+m, j] = sum_v sum_r T_v[r, m] * xpad[i0+r, j+v]
with T_v[r, m] = w[r-m, v] (band 0 <= r-m < 7). The 7 column taps (v)
are 7 matmuls accumulating into the same PSUM bank, each reading the
same SBUF x-tile at a shifted column offset. Inputs are cast to fp16
host-side (full-rate on the PE, ~1e-3 rel err); accumulation is fp32;
outputs are stored as bf16 and upcast on the host.

DMA strategy: one staging tile per image holds the 5 overlapping
128-row chunks (row stride 122 between chunks) so each image loads with
2 big DMAs; outputs collect into one [128,5,512] tile and store with 2
DMAs per image into a (p, chunk)-major DRAM layout so the writes walk
DRAM sequentially (strided interleaved writes measured ~25x slower).
The host un-permutes the chunked output. Loads and stores alternate
between the two HWDGE rings (sync / scalar) per image.
"""

import numpy as np

B, H, W = 128, 512, 512
KS = 7
PAD = (KS - 1) // 2          # 3
HP = H + 2 * PAD             # 518
N_CORES = 8
PER_CORE = B // N_CORES      # 16
MBLK = 128 - (KS - 1)        # 122 output rows per full block
NBLK = 5                     # ceil(512 / 122); last block has 24 rows
MRUNT = H - 4 * MBLK         # 24
KRUNT = HP - 4 * MBLK        # 30


def _build_program():
    import concourse.bass as bass
    import concourse.tile as tile
    from concourse import bacc, mybir

    f16 = mybir.dt.float16
    bf16 = mybir.dt.bfloat16
    f32 = mybir.dt.float32

    nc = bacc.Bacc("TRN2", target_bir_lowering=False, debug=False,
                   num_devices=N_CORES)
    x_ext = nc.declare_dram_parameter("x", [PER_CORE, HP, HP], f16,
                                      isOutput=False)
    t_ext = nc.declare_dram_parameter("toep", [128, KS * 128], f16,
                                      isOutput=False)
    # p-major chunked output: opc[img, p, c, :] = out row 122c + p
    opc_ext = nc.declare_dram_parameter("opc", [PER_CORE, MBLK, 4, W],
                                        bf16, isOutput=True)
    ort_ext = nc.declare_dram_parameter("ort", [PER_CORE, MRUNT, W],
                                        bf16, isOutput=True)

    with tile.TileContext(nc) as tc:
        with (
            tc.tile_pool(name="toep", bufs=1) as toep_pool,
            tc.tile_pool(name="xin", bufs=4) as x_pool,
            tc.tile_pool(name="psum", bufs=8, space="PSUM") as psum_pool,
            tc.tile_pool(name="outs", bufs=3) as out_pool,
        ):
            toep_sb = toep_pool.tile([128, KS * 128], f16)
            nc.sync.dma_start(out=toep_sb[:], in_=t_ext[:])

            for img in range(PER_CORE):
                # Stage the image as 5 overlapping 128-row chunks:
                # chunk c holds padded rows [122c, 122c+128).
                stage = x_pool.tile([128, NBLK, HP], f16)
                src = bass.AP(
                    x_ext,
                    img * HP * HP,
                    [(HP, 128), (MBLK * HP, 4), (1, HP)],
                )
                nc.sync.dma_start(out=stage[:, 0:4, :], in_=src)
                # chunk 4: padded rows [488, 518) (30 rows; rest unused)
                nc.sync.dma_start(out=stage[:KRUNT, 4, :],
                                  in_=x_ext[img, 4 * MBLK:HP, :])

                o_sb = out_pool.tile([128, NBLK, W], bf16)
                psums = [psum_pool.tile([128, W], f32, name=f"acc{b}",
                                        tag="acc") for b in range(NBLK)]
                # v-outer: all 5 blocks share one Toeplitz per tap, so the
                # stationary operand only changes every 5th matmul.
                for v in range(KS):
                    for b in range(NBLK):
                        kin = 128 if b < 4 else KRUNT
                        nc.tensor.matmul(
                            psums[b][:128, :],
                            toep_sb[:kin, v * 128:(v + 1) * 128],
                            stage[:kin, b, v:v + W],
                            start=(v == 0),
                            stop=(v == KS - 1),
                        )
                for b in range(NBLK):
                    m = MBLK if b < 4 else MRUNT
                    nc.vector.tensor_copy(o_sb[:m, b, :], psums[b][:m, :])

                nc.scalar.dma_start(out=opc_ext[img], in_=o_sb[:MBLK, 0:4, :])
                nc.scalar.dma_start(out=ort_ext[img], in_=o_sb[:MRUNT, 4, :])
    nc.finalize()
    return nc


def _host_prep(x, w):
    x = np.asarray(x, dtype=np.float32)
    w = np.asarray(w, dtype=np.float32)
    xpad = np.zeros((B, HP, HP), dtype=np.float16)
    xpad[:, PAD:PAD + H, PAD:PAD + W] = x
    # Toeplitz padded to 128 cols (cols >= MBLK are zero -> garbage out
    # rows that are never stored); 128 weight cols also enables FWL.
    toep = np.zeros((128, KS * 128), dtype=np.float16)
    w16 = w.astype(np.float16)
    idx = np.arange(MBLK)
    for v in range(KS):
        for d in range(KS):
            toep[idx + d, v * 128 + idx] = w16[d, v]
    return xpad, toep


def _execute(x, w, **run_kwargs):
    from concourse.bass_utils import run_bass_kernel_spmd

    xpad, toep = _host_prep(x, w)
    nc = _build_program()
    in_maps = [
        {"x": xpad[c * PER_CORE:(c + 1) * PER_CORE], "toep": toep}
        for c in range(N_CORES)
    ]
    res = run_bass_kernel_spmd(nc, in_maps, core_ids=list(range(N_CORES)),
                               **run_kwargs)
    out = np.empty((B, H, W), dtype=np.float32)
    for c in range(N_CORES):
        sl = slice(c * PER_CORE, (c + 1) * PER_CORE)
        opc = np.asarray(res.results[c]["opc"], dtype=np.float32)
        ort = np.asarray(res.results[c]["ort"], dtype=np.float32)
        # opc[img, p, c, :] -> rows (c*MBLK + p); c-major then p
        out[sl, :4 * MBLK, :] = opc.transpose(0, 2, 1, 3).reshape(
            PER_CORE, 4 * MBLK, W)
        out[sl, 4 * MBLK:, :] = ort
    return out, res


def kernel(x, w):
    out, _ = _execute(x, w)
    return out
